# revision 19
# baseline (speedup 1.0000x reference)
"""Trainium2 Bass kernel for ChunkAttentionMaskLayer.

Reference semantics (B=32, L=1024, N_SHIFT=10):
    out[b, i, j] = 1  iff  |i - j| <= 10  and  cumsum(mask)[b, i] == cumsum(mask)[b, j]
where cumsum is the inclusive prefix sum of chunk_st_mask along L
(equal prefix sums <=> no chunk start strictly between the two positions).

Strategy (pure data-parallel over batch, 4 batches per core), raw bacc with
hand-placed semaphores (no Tile tail barrier):
  * chunk ids: DVE tensor_tensor_scan over the [4, 1024] int32 mask (one
    batch per partition), split in two 512-column chunks chained via the
    scan's `initial` operand so downstream engines start on the first half
    early. Scan state is fp32 (exact for values <= 1024); a fp16 copy (cs16)
    feeds the PE (fp16 exact below 2048).
  * colvals cv[p, j] = cs[b, j]: fp16 selector matmuls on the PE
    (lhsT = "pick row b, broadcast to 128 partitions", rhs = cs16 chunk).
  * rowvals natqT[p, .] = cs[b, 128 t + p]: eight tiny [4,128]x[4,4] fp16
    matmuls against an identity (= transposes) into two single-bank PSUM
    tensors (lo: stripes 0-3, hi: 4-7) so DVE reads never share a PSUM bank
    with pending PE writes (concurrent PE-write + DVE-read of one PSUM bank
    is a hardware-fatal condition).
  * per 128-row stripe t, one fused DVE scalar_tensor_tensor computes
      band_out = (cv == rowval) * band_const
    over only the <=148 columns [128 t - 10, 128 t + 138) that can be nonzero.
  * only those band slices are DMAd out (3 DMAs per batch; the middle six
    stripes share one affine 3D access pattern). Everything else stays zero:
    run_bass_kernel_spmd pre-zeroes ExternalOutput buffers (the native path
    hands np.zeros to run_neff; the axon/PJRT path donates zero buffers),
    so the untouched 99.7% of the [B, L, L] output is already zero.
"""

import numpy as np

B, L = 32, 1024
NSHIFT = 10
NCORES = 8
BPC = B // NCORES  # batches per core
T = L // 128  # 128-row stripes per batch
WB = 128 + 2 * NSHIFT  # max nonzero band width per stripe (148)
HL = L // 2  # scan chunk length

_built = None
_last_results = None  # stashed BassKernelResults for test harnesses


def _host_consts():
    f16 = np.float16
    ident4 = np.eye(BPC, dtype=f16)
    # selb4[:, 128 b : 128 (b+1)] selects row b of cs16 and broadcasts it
    # across all 128 output partitions: selb4[c, 128 b + p] = (c == b)
    selb4 = np.kron(np.eye(BPC, dtype=f16), np.ones((1, 128), dtype=f16))
    # band[p, c] = 1 iff p <= c <= p + 20  (columns c index j - (128 t - 10))
    p = np.arange(128)[:, None]
    c = np.arange(WB)[None, :]
    band = ((c >= p) & (c <= p + 2 * NSHIFT)).astype(np.float32)
    return {"ident4": ident4, "selb4": selb4, "band": band}


def _build_program():
    from contextlib import ExitStack

    import concourse.bacc as bacc
    import concourse.bass as bass
    import concourse.mybir as mybir

    f32 = mybir.dt.float32
    f16 = mybir.dt.float16
    i32 = mybir.dt.int32
    Alu = mybir.AluOpType

    nc = bacc.Bacc(
        "TRN2", target_bir_lowering=False, debug=False, num_devices=NCORES
    )

    mask_t = nc.dram_tensor("mask", [BPC, L], i32, kind="ExternalInput")
    ident4_t = nc.dram_tensor("ident4", [BPC, BPC], f16, kind="ExternalInput")
    selb4_t = nc.dram_tensor("selb4", [BPC, BPC * 128], f16, kind="ExternalInput")
    band_t = nc.dram_tensor("band", [128, WB], f32, kind="ExternalInput")
    out_t = nc.dram_tensor("out", [BPC, L, L], i32, kind="ExternalOutput")
    out_ap = out_t.ap()

    def edges(t):
        c0 = max(0, 128 * t - NSHIFT)
        c1 = min(L, 128 * t + 128 + NSHIFT)
        return c0, c1 - c0

    # s_v counts: scan1=1, cast1=2, scan2=3, cast2=4, STT(b, t) = 5 + 8 b + t
    def sv_stt(b, t):
        return 5 + T * b + t

    # s_pe counts: tr_lo 1-4, mm(0, 0-3) 5-8, tr_hi 9-12, mm(0, 4-7) 13-16,
    # mm(b>=1, t) = 16 + 8 (b-1) + t + 1
    def spe_need(b, t):
        if b == 0:
            return 8 if t <= 2 else 16
        return 16 + T * (b - 1) + 4 if t <= 2 else 16 + T * b

    with ExitStack() as ctx:
        sb = lambda name, shape, dt: ctx.enter_context(
            nc.sbuf_tensor(name, shape, dt)
        )
        ps = lambda name, shape, dt: ctx.enter_context(
            nc.psum_tensor(name, shape, dt)
        )

        mask_i = sb("mask_i", [BPC, L], i32)
        cs = sb("cs", [BPC, L], f32)
        cs16 = sb("cs16", [BPC, L], f16)
        ident4 = sb("ident4_s", [BPC, BPC], f16)
        selb4 = sb("selb4_s", [BPC, BPC * 128], f16)
        band = sb("band_s", [128, WB], f32)
        sts = [sb(f"st{b}", [128, T * WB], i32) for b in range(BPC)]

        natqT_lo = ps("natqT_lo", [128, BPC * T // 2], f32)
        natqT_hi = ps("natqT_hi", [128, BPC * T // 2], f32)
        cvs = [ps(f"cv{i}", [128, L], f32) for i in range(2)]

        def rowval(b, t):
            if t < T // 2:
                return natqT_lo[:, BPC * t + b : BPC * t + b + 1]
            return natqT_hi[:, BPC * (t - T // 2) + b : BPC * (t - T // 2) + b + 1]

        s_mask = ctx.enter_context(nc.semaphore("s_mask"))
        s_id = ctx.enter_context(nc.semaphore("s_id"))
        s_sel = ctx.enter_context(nc.semaphore("s_sel"))
        s_band = ctx.enter_context(nc.semaphore("s_band"))
        s_v = ctx.enter_context(nc.semaphore("s_v"))
        s_pe = ctx.enter_context(nc.semaphore("s_pe"))
        s_out = ctx.enter_context(nc.semaphore("s_out"))

        block = ctx.enter_context(nc.Block())

        def out_dmas(eng, b):
            stv = sts[b][:].rearrange("p (t c) -> p t c", t=T)
            eng.dma_start(
                out_ap[b, 0:128, 0 : WB - NSHIFT], stv[:, 0, 0 : WB - NSHIFT]
            )._wait_ge(s_v, sv_stt(b, 0)).then_inc(s_out, 16)
            dst_mid = bass.AP(
                out_t,
                b * L * L + (128 * L + 128) - NSHIFT,
                [[L, 128], [128 * L + 128, T - 2], [1, WB]],
            )
            eng.dma_start(dst_mid, stv[:, 1 : T - 1, :])._wait_ge(
                s_v, sv_stt(b, T - 2)
            ).then_inc(s_out, 16)
            eng.dma_start(
                out_ap[b, 128 * (T - 1) : L, 128 * (T - 1) - NSHIFT : L],
                stv[:, T - 1, 0 : WB - NSHIFT],
            )._wait_ge(s_v, sv_stt(b, T - 1)).then_inc(s_out, 16)

        @block.sync
        def _(sync):
            sync.dma_start(ident4[:], ident4_t.ap()).then_inc(s_id, 16)
            sync.dma_start(selb4[:], selb4_t.ap()).then_inc(s_sel, 16)
            sync.dma_start(band[:], band_t.ap()).then_inc(s_band, 16)
            for b in range(0, BPC, 2):
                out_dmas(sync, b)
            sync.wait_ge(s_out, 16 * 3 * BPC)

        @block.scalar
        def _(scalar):
            # mask load issues here: the ACT engine finishes its preamble
            # earlier than SP, and the scan is the head of the critical path
            scalar.dma_start(mask_i[:], mask_t.ap()).then_inc(s_mask, 16)
            for b in range(1, BPC, 2):
                out_dmas(scalar, b)
            scalar.wait_ge(s_out, 16 * 3 * BPC)

        @block.tensor
        def _(tensor):
            tensor.wait_ge(s_id, 16)
            tensor.wait_ge(s_v, 2)  # scan1 + cast1
            for t in range(T // 2):  # rowvals for stripes 0-3
                nc.tensor.matmul(
                    natqT_lo[:, BPC * t : BPC * (t + 1)],
                    cs16[:, 128 * t : 128 * (t + 1)],
                    ident4[:],
                    start=True,
                    stop=True,
                ).then_inc(s_pe, 1)
            tensor.wait_ge(s_sel, 16)
            for t in range(T // 2):  # colvals batch 0, bank 0
                nc.tensor.matmul(
                    cvs[0][:, 128 * t : 128 * (t + 1)],
                    selb4[:, 0:128],
                    cs16[:, 128 * t : 128 * (t + 1)],
                    start=True,
                    stop=True,
                ).then_inc(s_pe, 1)
            tensor.wait_ge(s_v, 4)  # scan2 + cast2
            for t in range(T // 2, T):  # rowvals for stripes 4-7
                nc.tensor.matmul(
                    natqT_hi[:, BPC * (t - T // 2) : BPC * (t - T // 2 + 1)],
                    cs16[:, 128 * t : 128 * (t + 1)],
                    ident4[:],
                    start=True,
                    stop=True,
                ).then_inc(s_pe, 1)
            for t in range(T // 2, T):  # colvals batch 0, bank 1
                nc.tensor.matmul(
                    cvs[0][:, 128 * t : 128 * (t + 1)],
                    selb4[:, 0:128],
                    cs16[:, 128 * t : 128 * (t + 1)],
                    start=True,
                    stop=True,
                ).then_inc(s_pe, 1)
            for b in range(1, BPC):
                for t in range(T):
                    mm = nc.tensor.matmul(
                        cvs[b % 2][:, 128 * t : 128 * (t + 1)],
                        selb4[:, 128 * b : 128 * (b + 1)],
                        cs16[:, 128 * t : 128 * (t + 1)],
                        start=True,
                        stop=True,
                    )
                    if b >= 2 and t == 0:
                        # cv buffer reuse: all STTs of batch b-2 must be done
                        mm._wait_ge(s_v, sv_stt(b - 2, T - 1))
                    mm.then_inc(s_pe, 1)

        @block.vector
        def _(vector):
            vector.wait_ge(s_mask, 16)
            # inclusive prefix sum in two chained chunks; DVE converts the
            # int32 operands to its fp32 scan state
            nc.vector.tensor_tensor_scan(
                cs[:, 0:HL],
                mask_i[:, 0:HL],
                mask_i[:, 0:HL],
                0.0,
                op0=Alu.add,
                op1=Alu.bypass,
            ).then_inc(s_v, 1)
            nc.vector.tensor_copy(cs16[:, 0:HL], cs[:, 0:HL])._wait_ge(
                s_v, 1
            ).then_inc(s_v, 1)
            nc.vector.tensor_tensor_scan(
                cs[:, HL:L],
                mask_i[:, HL:L],
                mask_i[:, HL:L],
                cs[:, HL - 1 : HL],
                op0=Alu.add,
                op1=Alu.bypass,
            )._wait_ge(s_v, 1).then_inc(s_v, 1)
            nc.vector.tensor_copy(cs16[:, HL:L], cs[:, HL:L])._wait_ge(
                s_v, 3
            ).then_inc(s_v, 1)
            vector.wait_ge(s_band, 16)
            for b in range(BPC):
                for t in range(T):
                    c0, w = edges(t)
                    z0 = NSHIFT if t == 0 else 0
                    nc.vector.scalar_tensor_tensor(
                        out=sts[b][:, WB * t : WB * t + w],
                        in0=cvs[b % 2][:, c0 : c0 + w],
                        scalar=rowval(b, t),
                        in1=band[:, z0 : z0 + w],
                        op0=Alu.is_equal,
                        op1=Alu.mult,
                    )._wait_ge(s_pe, spe_need(b, t)).then_inc(s_v, 1)

    nc.compile()
    return nc


def kernel(chunk_st_mask: np.ndarray) -> np.ndarray:
    global _built, _last_results
    from concourse.bass_utils import run_bass_kernel_spmd

    if _built is None:
        _built = _build_program()
    nc = _built

    consts = _host_consts()
    chunk_st_mask = np.asarray(chunk_st_mask)
    in_maps = []
    for k in range(NCORES):
        shard = np.ascontiguousarray(
            chunk_st_mask[k * BPC : (k + 1) * BPC], dtype=np.int32
        )
        in_maps.append({"mask": shard, **consts})

    res = run_bass_kernel_spmd(nc, in_maps, core_ids=list(range(NCORES)))
    _last_results = res
    outs = [res.results[k]["out"].reshape(BPC, L, L) for k in range(NCORES)]
    return np.concatenate(outs, axis=0).astype(np.int32)


# revision 20
# speedup vs baseline: 1.0767x; 1.0767x over previous
"""Trainium2 Bass kernel for ChunkAttentionMaskLayer.

Reference semantics (B=32, L=1024, N_SHIFT=10):
    out[b, i, j] = 1  iff  |i - j| <= 10  and  cumsum(mask)[b, i] == cumsum(mask)[b, j]
where cumsum is the inclusive prefix sum of chunk_st_mask along L
(equal prefix sums <=> no chunk start strictly between the two positions).

Strategy (pure data-parallel over batch, 4 batches per core), raw bacc with
hand-placed semaphores (no Tile tail barrier):
  * chunk ids: DVE tensor_tensor_scan over the [4, 1024] int32 mask (one
    batch per partition), split in two 512-column chunks chained via the
    scan's `initial` operand so downstream engines start on the first half
    early. Scan state is fp32 (exact for values <= 1024); a fp16 copy (cs16)
    feeds the PE (fp16 exact below 2048).
  * colvals cv[p, j] = cs[b, j]: fp16 selector matmuls on the PE
    (lhsT = "pick row b, broadcast to 128 partitions", rhs = cs16 chunk).
  * rowvals natqT[p, .] = cs[b, 128 t + p]: eight tiny [4,128]x[4,4] fp16
    matmuls against an identity (= transposes) into two single-bank PSUM
    tensors (lo: stripes 0-3, hi: 4-7) so DVE reads never share a PSUM bank
    with pending PE writes (concurrent PE-write + DVE-read of one PSUM bank
    is a hardware-fatal condition).
  * per 128-row stripe t, one fused DVE scalar_tensor_tensor computes
      band_out = (cv == rowval) * band_const
    over only the <=148 columns [128 t - 10, 128 t + 138) that can be nonzero.
  * only those band slices are DMAd out (3 DMAs per batch; the middle six
    stripes share one affine 3D access pattern). Everything else stays zero:
    run_bass_kernel_spmd pre-zeroes ExternalOutput buffers (the native path
    hands np.zeros to run_neff; the axon/PJRT path donates zero buffers),
    so the untouched 99.7% of the [B, L, L] output is already zero.
"""

import numpy as np

B, L = 32, 1024
NSHIFT = 10
NCORES = 8
BPC = B // NCORES  # batches per core
T = L // 128  # 128-row stripes per batch
WB = 128 + 2 * NSHIFT  # max nonzero band width per stripe (148)
HL = L // 2  # scan chunk length

_built = None
_last_results = None  # stashed BassKernelResults for test harnesses


def _host_consts():
    f16 = np.float16
    ident4 = np.eye(BPC, dtype=f16)
    # selb4[:, 128 b : 128 (b+1)] selects row b of cs16 and broadcasts it
    # across all 128 output partitions: selb4[c, 128 b + p] = (c == b)
    selb4 = np.kron(np.eye(BPC, dtype=f16), np.ones((1, 128), dtype=f16))
    # band[p, c] = 1 iff p <= c <= p + 20  (columns c index j - (128 t - 10))
    p = np.arange(128)[:, None]
    c = np.arange(WB)[None, :]
    band = ((c >= p) & (c <= p + 2 * NSHIFT)).astype(np.float32)
    return {"ident4": ident4, "selb4": selb4, "band": band}


def _build_program():
    from contextlib import ExitStack

    import concourse.bacc as bacc
    import concourse.bass as bass
    import concourse.mybir as mybir

    f32 = mybir.dt.float32
    f16 = mybir.dt.float16
    i32 = mybir.dt.int32
    Alu = mybir.AluOpType

    nc = bacc.Bacc(
        "TRN2", target_bir_lowering=False, debug=False, num_devices=NCORES
    )

    mask_t = nc.dram_tensor("mask", [BPC, L], i32, kind="ExternalInput")
    ident4_t = nc.dram_tensor("ident4", [BPC, BPC], f16, kind="ExternalInput")
    selb4_t = nc.dram_tensor("selb4", [BPC, BPC * 128], f16, kind="ExternalInput")
    band_t = nc.dram_tensor("band", [128, WB], f32, kind="ExternalInput")
    out_t = nc.dram_tensor("out", [BPC, L, L], i32, kind="ExternalOutput")
    out_ap = out_t.ap()

    def edges(t):
        c0 = max(0, 128 * t - NSHIFT)
        c1 = min(L, 128 * t + 128 + NSHIFT)
        return c0, c1 - c0

    # s_v counts: scan1=1, cast1=2, scan2=3, cast2=4, STT(b, t) = 5 + 8 b + t
    def sv_stt(b, t):
        return 5 + T * b + t

    # s_pe counts: tr_lo 1-4, mm(0, 0-3) 5-8, tr_hi 9-12, mm(0, 4-7) 13-16,
    # mm(b>=1, t) = 16 + 8 (b-1) + t + 1
    def spe_need(b, t):
        return 2 * T + T * b

    with ExitStack() as ctx:
        sb = lambda name, shape, dt: ctx.enter_context(
            nc.sbuf_tensor(name, shape, dt)
        )
        ps = lambda name, shape, dt: ctx.enter_context(
            nc.psum_tensor(name, shape, dt)
        )

        mask_i = sb("mask_i", [BPC, L], i32)
        cs = sb("cs", [BPC, L], f32)
        cs16 = sb("cs16", [BPC, L], f16)
        ident4 = sb("ident4_s", [BPC, BPC], f16)
        selb4 = sb("selb4_s", [BPC, BPC * 128], f16)
        band = sb("band_s", [128, WB], f32)
        sts = [sb(f"st{b}", [128, T * WB], i32) for b in range(BPC)]

        natqT_p = ps("natqT_p", [128, BPC * T], f32)
        cvs = [ps(f"cv{i}", [128, L], f32) for i in range(2)]

        def rowval(b, t):
            return natqT_p[:, BPC * t + b : BPC * t + b + 1]

        s_mask = ctx.enter_context(nc.semaphore("s_mask"))
        s_id = ctx.enter_context(nc.semaphore("s_id"))
        s_sel = ctx.enter_context(nc.semaphore("s_sel"))
        s_band = ctx.enter_context(nc.semaphore("s_band"))
        s_v = ctx.enter_context(nc.semaphore("s_v"))
        s_pe = ctx.enter_context(nc.semaphore("s_pe"))
        s_out = ctx.enter_context(nc.semaphore("s_out"))

        block = ctx.enter_context(nc.Block())

        def out_dmas(eng, b):
            stv = sts[b][:].rearrange("p (t c) -> p t c", t=T)
            eng.dma_start(
                out_ap[b, 0:128, 0 : WB - NSHIFT], stv[:, 0, 0 : WB - NSHIFT]
            )._wait_ge(s_v, sv_stt(b, 0)).then_inc(s_out, 16)
            dst_mid = bass.AP(
                out_t,
                b * L * L + (128 * L + 128) - NSHIFT,
                [[L, 128], [128 * L + 128, T - 2], [1, WB]],
            )
            eng.dma_start(dst_mid, stv[:, 1 : T - 1, :])._wait_ge(
                s_v, sv_stt(b, T - 2)
            ).then_inc(s_out, 16)
            eng.dma_start(
                out_ap[b, 128 * (T - 1) : L, 128 * (T - 1) - NSHIFT : L],
                stv[:, T - 1, 0 : WB - NSHIFT],
            )._wait_ge(s_v, sv_stt(b, T - 1)).then_inc(s_out, 16)

        @block.sync
        def _(sync):
            sync.dma_start(mask_i[:], mask_t.ap()).then_inc(s_mask, 16)
            sync.dma_start(ident4[:], ident4_t.ap()).then_inc(s_id, 16)
            sync.dma_start(selb4[:], selb4_t.ap()).then_inc(s_sel, 16)
            sync.dma_start(band[:], band_t.ap()).then_inc(s_band, 16)
            for b in range(0, BPC, 2):
                out_dmas(sync, b)
            sync.wait_ge(s_out, 16 * 3 * BPC)

        @block.scalar
        def _(scalar):
            for b in range(1, BPC, 2):
                out_dmas(scalar, b)
            scalar.wait_ge(s_out, 16 * 3 * BPC)

        @block.tensor
        def _(tensor):
            tensor.wait_ge(s_id, 16)
            tensor.wait_ge(s_v, 2)  # scan1 + cast1
            for t in range(T // 2):  # rowvals for stripes 0-3
                nc.tensor.matmul(
                    natqT_p[:, BPC * t : BPC * (t + 1)],
                    cs16[:, 128 * t : 128 * (t + 1)],
                    ident4[:],
                    start=True,
                    stop=True,
                ).then_inc(s_pe, 1)
            tensor.wait_ge(s_sel, 16)
            for t in range(T // 2):  # colvals batch 0, bank 0
                nc.tensor.matmul(
                    cvs[0][:, 128 * t : 128 * (t + 1)],
                    selb4[:, 0:128],
                    cs16[:, 128 * t : 128 * (t + 1)],
                    start=True,
                    stop=True,
                ).then_inc(s_pe, 1)
            tensor.wait_ge(s_v, 4)  # scan2 + cast2
            for t in range(T // 2, T):  # rowvals for stripes 4-7
                nc.tensor.matmul(
                    natqT_p[:, BPC * t : BPC * (t + 1)],
                    cs16[:, 128 * t : 128 * (t + 1)],
                    ident4[:],
                    start=True,
                    stop=True,
                ).then_inc(s_pe, 1)
            for t in range(T // 2, T):  # colvals batch 0, bank 1
                nc.tensor.matmul(
                    cvs[0][:, 128 * t : 128 * (t + 1)],
                    selb4[:, 0:128],
                    cs16[:, 128 * t : 128 * (t + 1)],
                    start=True,
                    stop=True,
                ).then_inc(s_pe, 1)
            for b in range(1, BPC):
                for t in range(T):
                    mm = nc.tensor.matmul(
                        cvs[b % 2][:, 128 * t : 128 * (t + 1)],
                        selb4[:, 128 * b : 128 * (b + 1)],
                        cs16[:, 128 * t : 128 * (t + 1)],
                        start=True,
                        stop=True,
                    )
                    if b >= 2 and t == 0:
                        # cv buffer reuse: all STTs of batch b-2 must be done
                        mm._wait_ge(s_v, sv_stt(b - 2, T - 1))
                    mm.then_inc(s_pe, 1)

        @block.vector
        def _(vector):
            vector.wait_ge(s_mask, 16)
            # inclusive prefix sum in two chained chunks; DVE converts the
            # int32 operands to its fp32 scan state
            nc.vector.tensor_tensor_scan(
                cs[:, 0:HL],
                mask_i[:, 0:HL],
                mask_i[:, 0:HL],
                0.0,
                op0=Alu.add,
                op1=Alu.bypass,
            ).then_inc(s_v, 1)
            nc.vector.tensor_copy(cs16[:, 0:HL], cs[:, 0:HL])._wait_ge(
                s_v, 1
            ).then_inc(s_v, 1)
            nc.vector.tensor_tensor_scan(
                cs[:, HL:L],
                mask_i[:, HL:L],
                mask_i[:, HL:L],
                cs[:, HL - 1 : HL],
                op0=Alu.add,
                op1=Alu.bypass,
            )._wait_ge(s_v, 1).then_inc(s_v, 1)
            nc.vector.tensor_copy(cs16[:, HL:L], cs[:, HL:L])._wait_ge(
                s_v, 3
            ).then_inc(s_v, 1)
            vector.wait_ge(s_band, 16)
            for b in range(BPC):
                for t in range(T):
                    c0, w = edges(t)
                    z0 = NSHIFT if t == 0 else 0
                    nc.vector.scalar_tensor_tensor(
                        out=sts[b][:, WB * t : WB * t + w],
                        in0=cvs[b % 2][:, c0 : c0 + w],
                        scalar=rowval(b, t),
                        in1=band[:, z0 : z0 + w],
                        op0=Alu.is_equal,
                        op1=Alu.mult,
                    )._wait_ge(s_pe, spe_need(b, t)).then_inc(s_v, 1)

    nc.compile()
    return nc


def kernel(chunk_st_mask: np.ndarray) -> np.ndarray:
    global _built, _last_results
    from concourse.bass_utils import run_bass_kernel_spmd

    if _built is None:
        _built = _build_program()
    nc = _built

    consts = _host_consts()
    chunk_st_mask = np.asarray(chunk_st_mask)
    in_maps = []
    for k in range(NCORES):
        shard = np.ascontiguousarray(
            chunk_st_mask[k * BPC : (k + 1) * BPC], dtype=np.int32
        )
        in_maps.append({"mask": shard, **consts})

    res = run_bass_kernel_spmd(nc, in_maps, core_ids=list(range(NCORES)))
    _last_results = res
    outs = [res.results[k]["out"].reshape(BPC, L, L) for k in range(NCORES)]
    return np.concatenate(outs, axis=0).astype(np.int32)


# revision 21
# speedup vs baseline: 1.2079x; 1.1218x over previous
"""Trainium2 Bass kernel for ChunkAttentionMaskLayer.

Reference semantics (B=32, L=1024, N_SHIFT=10):
    out[b, i, j] = 1  iff  |i - j| <= 10  and  cumsum(mask)[b, i] == cumsum(mask)[b, j]
where cumsum is the inclusive prefix sum of chunk_st_mask along L
(equal prefix sums <=> no chunk start strictly between the two positions).

Strategy (pure data-parallel over batch, 4 batches per core), raw bacc with
hand-placed semaphores (no Tile tail barrier):
  * chunk ids: DVE tensor_tensor_scan over the [4, 1024] int32 mask (one
    batch per partition) straight to fp16 (exact: values <= 1024 < 2048),
    split in two 512-column chunks chained via the scan's fp32 `initial`
    state so the PE can start on the first half early.
  * colvals cv[p, j] = cs[b, j]: fp16 selector matmuls on the PE
    (lhsT = "pick row b, broadcast to 128 partitions", rhs = cs16 chunk)
    into PSUM (fp32, exact).
  * rowvals natqT[p, 4 t + b] = cs[b, 128 t + p]: eight tiny [4,128]x[4,4]
    fp16 matmuls against an identity (= transposes) into PSUM. All eight
    run before any DVE band op reads natqT, and the per-bank write/read
    windows of the cv tensors never overlap (concurrent PE-write +
    DVE-read of one PSUM bank is a hardware-fatal condition).
  * per 128-row stripe t, one fused DVE scalar_tensor_tensor computes
      band_out = (cv == rowval) * band_const
    over only the <=148 columns [128 t - 10, 128 t + 138) that can be
    nonzero.
  * only those band slices are DMAd out: 4 DMAs per batch (two edges, the
    middle six stripes as two affine 3D access patterns), interleaved over
    both HWDGE rings (SP + ACT) in readiness order so both rings drain
    concurrently. Everything else stays zero: run_bass_kernel_spmd
    pre-zeroes ExternalOutput buffers (the native path hands np.zeros to
    run_neff; the axon/PJRT path donates zero buffers), so the untouched
    99.7% of the [B, L, L] output is already zero.
"""

import numpy as np

B, L = 32, 1024
NSHIFT = 10
NCORES = 8
BPC = B // NCORES  # batches per core
T = L // 128  # 128-row stripes per batch
WB = 128 + 2 * NSHIFT  # max nonzero band width per stripe (148)
HL = L // 2  # scan chunk length

_built = None
_last_results = None  # stashed BassKernelResults for test harnesses


def _host_consts():
    f16 = np.float16
    ident4 = np.eye(BPC, dtype=f16)
    # selb4[:, 128 b : 128 (b+1)] selects row b of cs16 and broadcasts it
    # across all 128 output partitions: selb4[c, 128 b + p] = (c == b)
    selb4 = np.kron(np.eye(BPC, dtype=f16), np.ones((1, 128), dtype=f16))
    # band[p, c] = 1 iff p <= c <= p + 20  (columns c index j - (128 t - 10))
    p = np.arange(128)[:, None]
    c = np.arange(WB)[None, :]
    band = ((c >= p) & (c <= p + 2 * NSHIFT)).astype(np.float32)
    return {"ident4": ident4, "selb4": selb4, "band": band}


def _build_program():
    from contextlib import ExitStack

    import concourse.bacc as bacc
    import concourse.bass as bass
    import concourse.mybir as mybir

    f32 = mybir.dt.float32
    f16 = mybir.dt.float16
    i32 = mybir.dt.int32
    Alu = mybir.AluOpType

    nc = bacc.Bacc(
        "TRN2", target_bir_lowering=False, debug=False, num_devices=NCORES
    )

    mask_t = nc.dram_tensor("mask", [BPC, L], i32, kind="ExternalInput")
    ident4_t = nc.dram_tensor("ident4", [BPC, BPC], f16, kind="ExternalInput")
    selb4_t = nc.dram_tensor("selb4", [BPC, BPC * 128], f16, kind="ExternalInput")
    band_t = nc.dram_tensor("band", [128, WB], f32, kind="ExternalInput")
    out_t = nc.dram_tensor("out", [BPC, L, L], i32, kind="ExternalOutput")
    out_ap = out_t.ap()

    def edges(t):
        c0 = max(0, 128 * t - NSHIFT)
        c1 = min(L, 128 * t + 128 + NSHIFT)
        return c0, c1 - c0

    # s_v counts: scan1=1, scan2=2, STT(b, t) = 3 + 8 b + t
    def sv_stt(b, t):
        return 3 + T * b + t

    # s_pe counts: tr_lo 1-4, mm(0, 0-3) 5-8, tr_hi 9-12, mm(0, 4-7) 13-16,
    # mm(b>=1, t) = 16 + 8 (b-1) + t + 1.
    # A band read of stripe t touches cv columns [128 t - 10, 128 t + 138):
    # t <= 2 stays inside the cv bank written by mm(b, 0-3); t >= 3 also
    # touches the bank written by mm(b, 4-7). natqT writes all precede
    # s_pe = 12.
    def spe_need(b, t):
        if b == 0:
            return 12 if t <= 2 else 16
        return 16 + T * (b - 1) + 4 if t <= 2 else 16 + T * b

    with ExitStack() as ctx:
        sb = lambda name, shape, dt: ctx.enter_context(
            nc.sbuf_tensor(name, shape, dt)
        )
        ps = lambda name, shape, dt: ctx.enter_context(
            nc.psum_tensor(name, shape, dt)
        )

        mask_i = sb("mask_i", [BPC, L], i32)
        cs16 = sb("cs16", [BPC, L], f16)
        ident4 = sb("ident4_s", [BPC, BPC], f16)
        selb4 = sb("selb4_s", [BPC, BPC * 128], f16)
        band = sb("band_s", [128, WB], f32)
        sts = [sb(f"st{b}", [128, T * WB], i32) for b in range(BPC)]

        natqT_p = ps("natqT_p", [128, BPC * T], f32)
        cvs = [ps(f"cv{i}", [128, L], f32) for i in range(2)]

        def rowval(b, t):
            return natqT_p[:, BPC * t + b : BPC * t + b + 1]

        s_mask = ctx.enter_context(nc.semaphore("s_mask"))
        s_id = ctx.enter_context(nc.semaphore("s_id"))
        s_sel = ctx.enter_context(nc.semaphore("s_sel"))
        s_band = ctx.enter_context(nc.semaphore("s_band"))
        s_v = ctx.enter_context(nc.semaphore("s_v"))
        s_pe = ctx.enter_context(nc.semaphore("s_pe"))
        s_out = ctx.enter_context(nc.semaphore("s_out"))

        block = ctx.enter_context(nc.Block())

        # the four output DMAs of one batch, keyed by readiness order
        def dma_edge0(eng, b):
            stv = sts[b][:].rearrange("p (t c) -> p t c", t=T)
            eng.dma_start(
                out_ap[b, 0:128, 0 : WB - NSHIFT], stv[:, 0, 0 : WB - NSHIFT]
            )._wait_ge(s_v, sv_stt(b, 0)).then_inc(s_out, 16)

        def dma_mid(eng, b, lo, hi):  # stripes [lo, hi)
            stv = sts[b][:].rearrange("p (t c) -> p t c", t=T)
            dst = bass.AP(
                out_t,
                b * L * L + (128 * L + 128) * lo - NSHIFT,
                [[L, 128], [128 * L + 128, hi - lo], [1, WB]],
            )
            eng.dma_start(dst, stv[:, lo:hi, :])._wait_ge(
                s_v, sv_stt(b, hi - 1)
            ).then_inc(s_out, 16)

        def dma_edge7(eng, b):
            stv = sts[b][:].rearrange("p (t c) -> p t c", t=T)
            eng.dma_start(
                out_ap[b, 128 * (T - 1) : L, 128 * (T - 1) - NSHIFT : L],
                stv[:, T - 1, 0 : WB - NSHIFT],
            )._wait_ge(s_v, sv_stt(b, T - 1)).then_inc(s_out, 16)

        # readiness-ordered list of (emit_fn(eng)) covering all batches
        plan = []
        for b in range(BPC):
            plan.append(lambda eng, b=b: dma_edge0(eng, b))
            plan.append(lambda eng, b=b: dma_mid(eng, b, 1, 4))
            plan.append(lambda eng, b=b: dma_mid(eng, b, 4, T - 1))
            plan.append(lambda eng, b=b: dma_edge7(eng, b))

        @block.sync
        def _(sync):
            sync.dma_start(mask_i[:], mask_t.ap()).then_inc(s_mask, 16)
            sync.dma_start(ident4[:], ident4_t.ap()).then_inc(s_id, 16)
            sync.dma_start(selb4[:], selb4_t.ap()).then_inc(s_sel, 16)
            sync.dma_start(band[:], band_t.ap()).then_inc(s_band, 16)
            for i, emit in enumerate(plan):
                if i % 2 == 0:
                    emit(sync)
            sync.wait_ge(s_out, 16 * len(plan))

        @block.scalar
        def _(scalar):
            for i, emit in enumerate(plan):
                if i % 2 == 1:
                    emit(scalar)
            scalar.wait_ge(s_out, 16 * len(plan))

        @block.tensor
        def _(tensor):
            tensor.wait_ge(s_id, 16)
            tensor.wait_ge(s_v, 1)  # scan chunk 1
            for t in range(T // 2):  # rowvals for stripes 0-3
                nc.tensor.matmul(
                    natqT_p[:, BPC * t : BPC * (t + 1)],
                    cs16[:, 128 * t : 128 * (t + 1)],
                    ident4[:],
                    start=True,
                    stop=True,
                ).then_inc(s_pe, 1)
            tensor.wait_ge(s_sel, 16)
            for t in range(T // 2):  # colvals batch 0, bank 0
                nc.tensor.matmul(
                    cvs[0][:, 128 * t : 128 * (t + 1)],
                    selb4[:, 0:128],
                    cs16[:, 128 * t : 128 * (t + 1)],
                    start=True,
                    stop=True,
                ).then_inc(s_pe, 1)
            tensor.wait_ge(s_v, 2)  # scan chunk 2
            for t in range(T // 2, T):  # rowvals for stripes 4-7
                nc.tensor.matmul(
                    natqT_p[:, BPC * t : BPC * (t + 1)],
                    cs16[:, 128 * t : 128 * (t + 1)],
                    ident4[:],
                    start=True,
                    stop=True,
                ).then_inc(s_pe, 1)
            for t in range(T // 2, T):  # colvals batch 0, bank 1
                nc.tensor.matmul(
                    cvs[0][:, 128 * t : 128 * (t + 1)],
                    selb4[:, 0:128],
                    cs16[:, 128 * t : 128 * (t + 1)],
                    start=True,
                    stop=True,
                ).then_inc(s_pe, 1)
            for b in range(1, BPC):
                for t in range(T):
                    mm = nc.tensor.matmul(
                        cvs[b % 2][:, 128 * t : 128 * (t + 1)],
                        selb4[:, 128 * b : 128 * (b + 1)],
                        cs16[:, 128 * t : 128 * (t + 1)],
                        start=True,
                        stop=True,
                    )
                    if b >= 2 and t == 0:
                        # cv buffer reuse: all STTs of batch b-2 must be done
                        mm._wait_ge(s_v, sv_stt(b - 2, T - 1))
                    mm.then_inc(s_pe, 1)

        @block.vector
        def _(vector):
            vector.wait_ge(s_mask, 16)
            # inclusive prefix sum in two chained chunks, fp16 out (exact
            # below 2048); the DVE scan state is fp32 and converts the int32
            # operands on read
            nc.vector.tensor_tensor_scan(
                cs16[:, 0:HL],
                mask_i[:, 0:HL],
                mask_i[:, 0:HL],
                0.0,
                op0=Alu.add,
                op1=Alu.bypass,
            ).then_inc(s_v, 1)
            nc.vector.tensor_tensor_scan(
                cs16[:, HL:L],
                mask_i[:, HL:L],
                mask_i[:, HL:L],
                cs16[:, HL - 1 : HL],
                op0=Alu.add,
                op1=Alu.bypass,
            )._wait_ge(s_v, 1).then_inc(s_v, 1)
            vector.wait_ge(s_band, 16)
            for b in range(BPC):
                for t in range(T):
                    c0, w = edges(t)
                    z0 = NSHIFT if t == 0 else 0
                    nc.vector.scalar_tensor_tensor(
                        out=sts[b][:, WB * t : WB * t + w],
                        in0=cvs[b % 2][:, c0 : c0 + w],
                        scalar=rowval(b, t),
                        in1=band[:, z0 : z0 + w],
                        op0=Alu.is_equal,
                        op1=Alu.mult,
                    )._wait_ge(s_pe, spe_need(b, t)).then_inc(s_v, 1)

    nc.compile()
    return nc


def kernel(chunk_st_mask: np.ndarray) -> np.ndarray:
    global _built, _last_results
    from concourse.bass_utils import run_bass_kernel_spmd

    if _built is None:
        _built = _build_program()
    nc = _built

    consts = _host_consts()
    chunk_st_mask = np.asarray(chunk_st_mask)
    in_maps = []
    for k in range(NCORES):
        shard = np.ascontiguousarray(
            chunk_st_mask[k * BPC : (k + 1) * BPC], dtype=np.int32
        )
        in_maps.append({"mask": shard, **consts})

    res = run_bass_kernel_spmd(nc, in_maps, core_ids=list(range(NCORES)))
    _last_results = res
    outs = [res.results[k]["out"].reshape(BPC, L, L) for k in range(NCORES)]
    return np.concatenate(outs, axis=0).astype(np.int32)
